# revision 14
# baseline (speedup 1.0000x reference)
"""Conv2D 3x3 (stride 1, pad 1) Trainium2 Bass kernel.

Problem: x (16,128,56,56) f32  *  W (256,128,3,3) f32  + b (256,)  ->  (16,256,56,56) f32

Strategy:
  - Data parallel over batch: 8 NeuronCores x 2 images each; W/b replicated.
  - Host pre-pads each image with a 1-pixel zero halo (58x58) so every kernel
    tap is a pure strided SBUF read -- no edge cases on device.
  - Implicit GEMM: contraction over C_IN=128 (the SBUF partition dim); for each
    output tile of 8 rows (N = 8*56 = 448 pixels, one PSUM bank) accumulate the
    9 taps as 9 matmuls: psum[co,pix] += W[ci,co,tap].T @ xpad[ci,shifted pix].
  - bf16 inputs (tolerance 2e-2 >> bf16 conv err ~2.4e-3): enables the PE's
    fast weight load (FWL, ~27ns vs ~195ns for fp32) and halves input DMA.
  - Chunk-major weight layout + split x DMAs so the first matmul only waits on
    ~0.5 MB of DMA; dummy warmup matmuls keep the PE HAM window busy meanwhile.
  - PSUM -> SBUF eviction + per-channel bias via one ScalarE activation.
"""

import os
import sys

for _p in ("/opt/trn_rl_repo", os.path.expanduser("~/.axon_site/_ro/trn_rl_repo")):
    if os.path.isdir(_p) and _p not in sys.path:
        sys.path.insert(0, _p)
        break

import numpy as np
import ml_dtypes

B, C_IN, H, W_SP = 16, 128, 56, 56
C_OUT, KH, KW = 256, 3, 3
N_CORES = 8
B_PER_CORE = B // N_CORES          # 2
CHUNKS = C_OUT // 128              # 2 chunks of 128 output channels
HP, WP = H + 2, W_SP + 2           # 58x58 padded image
ROWS_PER_TILE = 8                  # 8*56 = 448 <= 512 fp32 / PSUM bank
N_TILE = ROWS_PER_TILE * W_SP      # 448
H_TILES = H // ROWS_PER_TILE       # 7
W_PER_CHUNK = KH * KW * 128        # 1152 weight columns per chunk
N_WARM = 28                        # small-N dummy matmuls to pre-warm the PE clock

# x DMA piece boundaries (padded rows). Tile ht needs rows ht*8 .. ht*8+9.
# First image gets a finer split so the first matmul group's rows land early.
X_ROW_SPLITS0 = (0, 10, 16, 30, 44, HP)
X_ROW_SPLITS = (0, 16, 30, 44, HP)

_CACHE = {}


def _build(repeat=1, tag=0, null=False):
    from concourse import bacc, mybir
    import concourse.tile as tile

    f32 = mybir.dt.float32
    bf16 = mybir.dt.bfloat16

    nc = bacc.Bacc(trn_type="TRN2", name="conv2d_dp")
    x_h = nc.dram_tensor("x", [B_PER_CORE, C_IN, HP * WP], bf16, kind="ExternalInput")
    # wt layout: [ci, chunk*1152 + (kh*3+kw)*128 + co_mod]  (chunk-major so the
    # first chunk's weights land before the second's)
    # `tag` pads the wt free dim so benchmark variants hash differently in the
    # (BIR-payload-blind) neuron compile cache.
    w_h = nc.dram_tensor("wt", [C_IN, CHUNKS * W_PER_CHUNK + tag], bf16,
                         kind="ExternalInput")
    # bias layout: [co_mod, chunk]
    b_h = nc.dram_tensor("bias", [128, CHUNKS], f32, kind="ExternalInput")
    o_h = nc.dram_tensor("out", [B_PER_CORE, C_OUT, H, W_SP], f32, kind="ExternalOutput")

    with tile.TileContext(nc) as tc:
        with tc.tile_pool(name="const", bufs=1) as cpool, \
             tc.tile_pool(name="xs", bufs=2) as xpool, \
             tc.tile_pool(name="os", bufs=6) as opool, \
             tc.tile_pool(name="ps", bufs=8, space="PSUM") as ppool:
            b_sb = cpool.tile([128, CHUNKS], f32)
            nc.sync.dma_start(out=b_sb[:, :], in_=b_h[:, :])

            if null:
                # timing-overhead probe: same I/O signature, near-zero work
                zt = cpool.tile([128, N_TILE], f32)
                nc.vector.memset(zt[:, :], 0)
                nc.sync.dma_start(out=o_h[0, :128, :ROWS_PER_TILE, :], in_=zt[:, :])
                nc.finalize()
                return nc

            # PE warmup: small-N dummy matmuls on a zeroed tile into a scratch
            # PSUM bank, runnable right after the start barrier (no DMA
            # dependency). They overlap the initial weight/x DMA and open the
            # HAM activity window early; N=128 keeps the bridge granularity
            # fine so real matmuls start within ~100ns of their data landing.
            warm = cpool.tile([128, 128], bf16)
            nc.gpsimd.memset(warm[:, :], 0)
            wps = ppool.tile([128, N_TILE], f32, name="ps")
            for _ in range(N_WARM):
                nc.tensor.matmul(wps[:, :128], warm[:, :], warm[:, :],
                                 start=True, stop=True)

            # weights: chunk 0 in tap-triples (the first matmuls only need the
            # first taps), chunk 1 as one transfer
            w_sb = cpool.tile([C_IN, CHUNKS * W_PER_CHUNK], bf16)
            for s0, s1 in ((0, 3 * 128), (3 * 128, 6 * 128), (6 * 128, W_PER_CHUNK),
                           (W_PER_CHUNK, 2 * W_PER_CHUNK)):
                nc.sync.dma_start(out=w_sb[:, s0:s1], in_=w_h[:, s0:s1])

            # DMA trigger descriptors cost ~0.7us on the issuing engine queue
            # (only Sync/GpSimd/Scalar may issue DMAs); put x-piece triggers on
            # the otherwise-idle GpSimd queue so they don't serialize behind
            # the weight DMAs on Sync.
            x_qs = (nc.gpsimd, nc.gpsimd)
            o_qs = (nc.sync, nc.gpsimd, nc.sync)

            for rep in range(repeat):
              for b in range(B_PER_CORE):
                xp = xpool.tile([C_IN, HP * WP], bf16, name=f"xp{b}")
                splits = X_ROW_SPLITS0 if b == 0 else X_ROW_SPLITS
                for pi, (r0, r1) in enumerate(zip(splits, splits[1:])):
                    x_qs[pi % 2].dma_start(out=xp[:, r0 * WP:r1 * WP],
                                           in_=x_h[b, :, r0 * WP:r1 * WP])
                x3 = xp.rearrange("p (r c) -> p r c", r=HP)

                for chunk in range(CHUNKS):
                    for ht in range(H_TILES):
                        h0 = ht * ROWS_PER_TILE
                        ps = ppool.tile([128, N_TILE], f32, name="ps")
                        for tap in range(KH * KW):
                            dh, dw = divmod(tap, KW)
                            wcol = chunk * W_PER_CHUNK + tap * 128
                            nc.tensor.matmul(
                                ps[:, :],
                                w_sb[:, wcol:wcol + 128],
                                x3[:, h0 + dh:h0 + dh + ROWS_PER_TILE, dw:dw + W_SP],
                                start=(tap == 0),
                                stop=(tap == KH * KW - 1),
                            )
                        osb = opool.tile([128, N_TILE], f32, name="osb")
                        last = (b == B_PER_CORE - 1 and chunk == CHUNKS - 1
                                and ht == H_TILES - 1)
                        if last:
                            # split the final eviction so its PSUM->SBUF copy
                            # and DMA pipeline (shortens the kernel tail)
                            half = N_TILE // 2
                            for hi, q in ((0, nc.sync), (1, nc.gpsimd)):
                                nc.scalar.activation(
                                    osb[:, hi * half:(hi + 1) * half],
                                    ps[:, hi * half:(hi + 1) * half],
                                    mybir.ActivationFunctionType.Identity,
                                    bias=b_sb[:, chunk:chunk + 1],
                                )
                                q.dma_start(
                                    out=o_h[b, chunk * 128:(chunk + 1) * 128,
                                            h0 + hi * ROWS_PER_TILE // 2:
                                            h0 + (hi + 1) * ROWS_PER_TILE // 2, :],
                                    in_=osb[:, hi * half:(hi + 1) * half],
                                )
                        else:
                            nc.scalar.activation(
                                osb[:, :], ps[:, :],
                                mybir.ActivationFunctionType.Identity,
                                bias=b_sb[:, chunk:chunk + 1],
                            )
                            o_qs[(b * CHUNKS * H_TILES + chunk * H_TILES + ht) % 3].dma_start(
                                out=o_h[b, chunk * 128:(chunk + 1) * 128,
                                        h0:h0 + ROWS_PER_TILE, :],
                                in_=osb[:, :],
                            )
    nc.finalize()
    return nc


def _get_nc(repeat=1, tag=0, null=False):
    key = ("nc", repeat, tag, null)
    if key not in _CACHE:
        _CACHE[key] = _build(repeat, tag=tag, null=null)
    return _CACHE[key]


def kernel(x, W, b, _trace=False):
    from concourse.bass_utils import run_bass_kernel_spmd

    x = np.asarray(x, dtype=np.float32)
    W = np.asarray(W, dtype=np.float32)
    b = np.asarray(b, dtype=np.float32)

    bf16 = ml_dtypes.bfloat16
    # zero-pad spatial dims to 58x58 on host, flatten, cast to bf16
    xpad = np.zeros((B, C_IN, HP, WP), dtype=bf16)
    xpad[:, :, 1:1 + H, 1:1 + W_SP] = x.astype(bf16)
    xpad = np.ascontiguousarray(xpad.reshape(B, C_IN, HP * WP))

    # [co,ci,kh,kw] -> [ci, chunk*1152 + (kh*3+kw)*128 + co_mod]
    wt = np.ascontiguousarray(
        W.reshape(CHUNKS, 128, C_IN, KH, KW).transpose(2, 0, 3, 4, 1)
        .reshape(C_IN, CHUNKS * W_PER_CHUNK).astype(bf16))
    bias = np.ascontiguousarray(b.reshape(CHUNKS, 128).T)

    nc = _get_nc()
    in_maps = [
        {"x": xpad[c * B_PER_CORE:(c + 1) * B_PER_CORE], "wt": wt, "bias": bias}
        for c in range(N_CORES)
    ]
    res = run_bass_kernel_spmd(nc, in_maps, core_ids=list(range(N_CORES)),
                               trace=_trace)
    out = np.concatenate([res.results[c]["out"] for c in range(N_CORES)], axis=0)
    if _trace:
        _CACHE["last_results"] = res
    return out
